# revision 48
# baseline (speedup 1.0000x reference)
"""Trainium2 Bass kernel for DirectionAwareMessagePassing (gnn_message_passing).

Sharding: data-parallel over batch B=32 across 8 NeuronCores (4 graphs/core),
weights replicated. Host pre-computes transposed fp8 obj, one-hot gather /
scatter matrices and bf16 union so the device pipeline is pure matmul +
drain work:
  OS/OO/feat fp8 projections -> one-hot gather matmuls -> P.T = S.T*O.T ->
  Q = P @ (wu*w).T -> coeff = rowsum(union*Q)+w_b -> A scatter-matmul ->
  sigmoid/mask/row-normalize -> direction-aware ctx -> LN MLP ->
  residual (identity-matmul) relu.
Emission is software-pipelined at sub-graph granularity: the tail of graph g
is split so its serial LN/sigmoid chains overlap the next graph's matmuls.
"""

import sys

import numpy as np

if "/opt/trn_rl_repo" not in sys.path:
    sys.path.insert(0, "/opt/trn_rl_repo")

from concourse import bacc, mybir, tile
from concourse import bass_utils

import ml_dtypes

BF16 = ml_dtypes.bfloat16
FP8 = ml_dtypes.float8_e4m3

B, N, R, D = 32, 256, 2048, 1024
D2 = D // 2   # 512 feat dim
DQ = D // 4   # 256 LN dim
NCORES = 8
GPC = B // NCORES  # graphs per core
NT = N // 128      # 2 i-tiles
RT = R // 128      # 16 r-tiles
DT = D // 128      # 8 d-tiles
LN_EPS = 1e-5

f32 = mybir.dt.float32
bf16 = mybir.dt.bfloat16
fp8 = mybir.dt.float8e4
Alu = mybir.AluOpType
Act = mybir.ActivationFunctionType
DR = mybir.MatmulPerfMode.DoubleRow


def _build_fast():
    nc = bacc.Bacc("TRN2")

    # ---- DRAM tensors (per core) ----
    objbf_d = nc.dram_tensor("objbf", [GPC, N, D], bf16, kind="ExternalInput").ap()
    objT8_d = nc.dram_tensor("objT8", [GPC, 128, DT // 2, 2, N], fp8,
                             kind="ExternalInput").ap()
    un_d = nc.dram_tensor("unbf", [GPC, R, D], bf16, kind="ExternalInput").ap()
    esT8_d = nc.dram_tensor("esT8", [GPC, 2, 128, 2, R], fp8,
                            kind="ExternalInput").ap()
    esrow_d = nc.dram_tensor("esrow", [GPC, 128, RT // 2, 2, N], fp8,
                             kind="ExternalInput").ap()
    ohrow_d = nc.dram_tensor("ohrow", [GPC, 128, RT // 2, 2, N], fp8,
                             kind="ExternalInput").ap()
    ws8_d = nc.dram_tensor("ws8", [DT // 2, 128, 2, D], fp8, kind="ExternalInput").ap()
    wo8_d = nc.dram_tensor("wo8", [DT // 2, 128, 2, D], fp8, kind="ExternalInput").ap()
    wu8_d = nc.dram_tensor("wu8", [DT // 2, 128, 2, D], fp8, kind="ExternalInput").ap()
    t38_d = nc.dram_tensor("t38", [DT // 2, 128, 2, D2], fp8, kind="ExternalInput").ap()
    tr1_d = nc.dram_tensor("tr1bf", [D, DQ], bf16, kind="ExternalInput").ap()
    tr2_d = nc.dram_tensor("tr2bf", [DQ, D], bf16, kind="ExternalInput").ap()
    wb_d = nc.dram_tensor("wb", [1, 1], f32, kind="ExternalInput").ap()
    out_d = nc.dram_tensor("out", [GPC, N, D], f32, kind="ExternalOutput").ap()

    with tile.TileContext(nc) as tc:
        with tc.tile_pool(name="wpool", bufs=1) as wpool, \
             tc.tile_pool(name="cpool", bufs=1) as cpool, \
             tc.tile_pool(name="gpool", bufs=1) as gpool, \
             tc.tile_pool(name="spool", bufs=2) as spool, \
             tc.tile_pool(name="upool", bufs=4) as upool, \
             tc.tile_pool(name="mmps", bufs=4, space="PSUM") as mmps, \
             tc.tile_pool(name="qps_pool", bufs=2, space="PSUM") as qps_pool, \
             tc.tile_pool(name="tps_pool", bufs=1, space="PSUM") as tps_pool, \
             tc.tile_pool(name="aps_pool", bufs=1, space="PSUM") as aps_pool:

            # ========== startup DMA order: first-needed tensors first =========
            def load_proj_inputs(g):
                d = {}
                objT8 = gpool.tile([128, DT // 2, 2, N], fp8, name="objT8",
                                   tag="objT8", bufs=2)
                nc.sync.dma_start(objT8[:, :, :, :], objT8_d[g, :, :, :, :])
                d["objT8"] = objT8
                return d

            def load_mid_inputs(g, d):
                esT8 = []
                for s in range(2):
                    e8 = gpool.tile([128, 2, R], fp8, name=f"esT8{s}",
                                    tag=f"esT8{s}", bufs=2)
                    nc.sync.dma_start(e8[:, :, :], esT8_d[g, s, :, :, :])
                    esT8.append(e8)
                d["esT8"] = esT8
                esrow = gpool.tile([128, RT // 2, 2, N], fp8, name="esrow",
                                   tag="esrow", bufs=2)
                nc.sync.dma_start(esrow[:, :, :, :], esrow_d[g, :, :, :, :])
                d["esrow"] = esrow
                ohrow = gpool.tile([128, RT // 2, 2, N], fp8, name="ohrow",
                                   tag="ohrow", bufs=2)
                nc.sync.dma_start(ohrow[:, :, :, :], ohrow_d[g, :, :, :, :])
                d["ohrow"] = ohrow
                obj_bf = []
                for it in range(NT):
                    ob = gpool.tile([128, D], bf16, name=f"objbf{it}",
                                    tag=f"objbf{it}", bufs=2)
                    nc.sync.dma_start(ob[:, :], objbf_d[g, it * 128:(it + 1) * 128, :])
                    obj_bf.append(ob)
                d["obj_bf"] = obj_bf
                return d

            g0 = load_proj_inputs(0)

            def load_w8(dram, cols, name):
                tiles = []
                for t in range(DT // 2):
                    w8 = wpool.tile([128, 2, cols], fp8, name=f"{name}{t}",
                                    tag=f"{name}{t}")
                    nc.sync.dma_start(w8[:, :, :], dram[t, :, :, :])
                    tiles.append(w8)
                return tiles

            ws8_sb = load_w8(ws8_d, D, "ws8")
            wo8_sb = load_w8(wo8_d, D, "wo8")
            t38_sb = load_w8(t38_d, D2, "t38")
            g0 = load_mid_inputs(0, g0)
            wu8_sb = load_w8(wu8_d, D, "wu8")
            tr1_sb = []
            for t in range(DT):
                w = wpool.tile([128, DQ], bf16, name=f"tr1{t}", tag=f"tr1{t}")
                nc.sync.dma_start(w[:, :], tr1_d[t * 128:(t + 1) * 128, :])
                tr1_sb.append(w)
            tr2_sb = []
            for t in range(DQ // 128):
                w = wpool.tile([128, D], bf16, name=f"tr2{t}", tag=f"tr2{t}")
                nc.sync.dma_start(w[:, :], tr2_d[t * 128:(t + 1) * 128, :])
                tr2_sb.append(w)
            wb_p0 = cpool.tile([1, 1], f32, name="wb_p0", tag="wb_p0")
            nc.sync.dma_start(wb_p0[:, :], wb_d[:, :])
            wb_col = cpool.tile([128, 1], f32, name="wb_col", tag="wb_col")
            nc.gpsimd.partition_broadcast(wb_col[:, :], wb_p0[:, :])

            # ================= device constants =================
            ones_bf16 = cpool.tile([128, N], bf16, name="ones_bf16", tag="ones_bf16")
            nc.vector.memset(ones_bf16[:, :], 1.0)

            ident_bf16 = cpool.tile([128, 128], bf16, name="ident_bf16",
                                    tag="ident_bf16")
            nc.gpsimd.affine_select(
                ident_bf16[:, :], ones_bf16[:, :128], pattern=[[1, 128]],
                compare_op=Alu.is_equal, fill=0.0, base=0, channel_multiplier=-1)
            eyemask = []
            for it in range(NT):
                em = cpool.tile([128, N], bf16, name=f"eyemask{it}", tag=f"eyemask{it}")
                nc.gpsimd.affine_select(
                    em[:, :], ones_bf16[:, :], pattern=[[1, N]],
                    compare_op=Alu.not_equal, fill=0.0,
                    base=-(it * 128), channel_multiplier=-1)
                eyemask.append(em)
            eps_col = cpool.tile([128, 1], f32, name="eps_col", tag="eps_col")
            nc.vector.memset(eps_col[:, :], LN_EPS)

            # ================= per-graph pieces =================
            NCH = 4
            RCW = R // NCH            # r per chunk (512)
            RTC = RCW // 128          # r-tiles per chunk

            def make_head_steps(g, d):
                """Head (OS/OO/feat projections) as a list of emit-closures so
                they can be dripped into the previous graph's last chunk."""
                objT8 = d["objT8"]
                OS8 = gpool.tile([128, NT, D], fp8, name="OS8", tag="OS8", bufs=2)
                OO8 = gpool.tile([128, NT, D], fp8, name="OO8", tag="OO8", bufs=2)
                d["OS8"] = OS8
                d["OO8"] = OO8
                d["feat"] = []
                steps = []
                for dst3, w8_sb in ((OS8, ws8_sb), (OO8, wo8_sb)):
                    for it in range(NT):
                        for fc in range(2):
                            def s(dst3=dst3, w8_sb=w8_sb, it=it, fc=fc):
                                ps = mmps.tile([128, 512], f32, name="ps",
                                               tag="mm")
                                for dtp in range(DT // 2):
                                    nc.tensor.matmul(
                                        ps[:, :],
                                        objT8[:, dtp, :, it * 128:(it + 1) * 128],
                                        w8_sb[dtp][:, :, fc * 512:(fc + 1) * 512],
                                        perf_mode=DR,
                                        start=(dtp == 0),
                                        stop=(dtp == DT // 2 - 1))
                                nc.scalar.activation(
                                    dst3[:, it, fc * 512:(fc + 1) * 512],
                                    ps[:, :], Act.Copy, scale=1.0 / 64.0)
                            steps.append(s)
                for it in range(NT):
                    def s(it=it):
                        fps = mmps.tile([128, D2], f32, name="fps", tag="mm")
                        for dtp in range(DT // 2):
                            nc.tensor.matmul(
                                fps[:, :],
                                objT8[:, dtp, :, it * 128:(it + 1) * 128],
                                t38_sb[dtp][:, :, :],
                                perf_mode=DR,
                                start=(dtp == 0), stop=(dtp == DT // 2 - 1))
                        ft = gpool.tile([128, D2], bf16, name=f"feat{it}",
                                        tag=f"feat{it}", bufs=2)
                        nc.scalar.activation(ft[:, :], fps[:, :], Act.Relu,
                                             scale=1.0 / 64.0)
                        d["feat"].append(ft)
                    steps.append(s)
                return steps

            def emit_head(g, d):
                for s in make_head_steps(g, d):
                    s()
                return d

            def emit_gather_pair(g, hd, rc, dtp):
                """Two dt-gathers (one PT8 dtp tile's worth) of chunk rc."""
                OS8, OO8, esT8 = hd["OS8"], hd["OO8"], hd["esT8"]
                PT8 = hd["PT8"][rc % 2]
                fc = rc  # RCW == 512: one 512-chunk per rc
                for h in range(2):
                    dt = 2 * dtp + h
                    sps = mmps.tile([128, 512], f32, name="sps", tag="mm")
                    ops = mmps.tile([128, 512], f32, name="ops", tag="mm")
                    nc.tensor.matmul(
                        sps[:, :], OS8[:, :, dt * 128:(dt + 1) * 128],
                        esT8[0][:, :, fc * 512:(fc + 1) * 512],
                        perf_mode=DR, start=True, stop=True)
                    nc.tensor.matmul(
                        ops[:, :], OO8[:, :, dt * 128:(dt + 1) * 128],
                        esT8[1][:, :, fc * 512:(fc + 1) * 512],
                        perf_mode=DR, start=True, stop=True)
                    st_sb = spool.tile([128, 512], bf16, name="st_sb",
                                       tag="st_sb", bufs=3)
                    nc.scalar.copy(st_sb[:, :], sps[:, :])
                    nc.vector.scalar_tensor_tensor(
                        PT8[dt // 2][:, dt % 2, :],
                        ops[:, :], 16.0, st_sb[:, :],
                        op0=Alu.mult, op1=Alu.mult)

            def emit_gathers(g, hd, rc):
                pts = []
                for dtp in range(DT // 2):
                    pt = gpool.tile([128, 2, RCW], fp8, name=f"PT8{dtp}",
                                    tag=f"PT8{dtp}", bufs=2)
                    pts.append(pt)
                hd.setdefault("PT8", {})[rc % 2] = pts
                for dtp in range(DT // 2):
                    emit_gather_pair(g, hd, rc, dtp)

            def emit_qloop(g, hd, rc, gath_next, head_steps=None):
                """Q/coeff/eoc/A-scatter for chunk rc; interleaves the NEXT
                chunk's gathers so every engine queue stays just-in-time."""
                esrow, ohrow = hd["esrow"], hd["ohrow"]
                if rc == 0:
                    hd["coeff"] = gpool.tile([128, RT], f32, name="coeff",
                                             tag="coeff", bufs=2)
                    hd["A_ps"] = aps_pool.tile([128, 2 * N], f32, name="A_ps",
                                               tag="A_ps")
                coeff, A_ps = hd["coeff"], hd["A_ps"]
                PT8 = hd["PT8"][rc % 2]
                if gath_next is not None:
                    nxt_pts = []
                    for dtp in range(DT // 2):
                        pt = gpool.tile([128, 2, RCW], fp8, name=f"PT8{dtp}",
                                        tag=f"PT8{dtp}", bufs=2)
                        nxt_pts.append(pt)
                    hd["PT8"][gath_next % 2] = nxt_pts
                if rc == 2 and g == GPC - 1:
                    # last graph: preload the sigmoid table off the end chain
                    dum = spool.tile([128, 1], f32, name="dsig", tag="dsig")
                    nc.scalar.activation(dum[:, :], eps_col[:, :], Act.Sigmoid)
                un2 = upool.tile([128, RTC, D], bf16, name="un2", tag="un",
                                 bufs=3)
                nc.sync.dma_start(un2[:, :, :],
                                  un_d[g, rc * RCW:(rc + 1) * RCW, :])
                for rpl in range(RTC // 2):
                    rp = rc * (RTC // 2) + rpl   # rt-pair index
                    # eoc pair [128, 2, N] fp8: coeff (x32) folded per rt slice
                    eoc = spool.tile([128, 2, N], fp8, name="eoc", tag="eoc",
                                      bufs=3)
                    for j in range(2):
                        rtl = rpl * 2 + j
                        rt = rc * RTC + rtl
                        accs = []
                        for fc in range(2):
                            qp = qps_pool.tile([128, 512], f32, name="qps",
                                               tag="qps")
                            for dtp in range(DT // 2):
                                nc.tensor.matmul(
                                    qp[:, :],
                                    PT8[dtp][:, :, rtl * 128:(rtl + 1) * 128],
                                    wu8_sb[dtp][:, :, fc * 512:(fc + 1) * 512],
                                    perf_mode=DR,
                                    start=(dtp == 0), stop=(dtp == DT // 2 - 1))
                            junk = spool.tile([128, 512], bf16, name="junk",
                                              tag="junk", bufs=3)
                            acc = spool.tile([128, 1], f32, name=f"acc{fc}",
                                             tag=f"acc{fc}")
                            nc.vector.scalar_tensor_tensor(
                                junk[:, :], qp[:, :], 32.0 / 65536.0,
                                un2[:, rtl, fc * 512:(fc + 1) * 512],
                                op0=Alu.mult, op1=Alu.mult, accum_out=acc[:, :])
                            accs.append(acc)
                        nc.vector.tensor_tensor(coeff[:, rt:rt + 1],
                                                accs[0][:, :], accs[1][:, :],
                                                op=Alu.add)
                        nc.scalar.activation(eoc[:, j, :], ohrow[:, rp, j, :],
                                             Act.Copy,
                                             scale=coeff[:, rt:rt + 1])
                        if gath_next is not None:
                            emit_gather_pair(g, hd, gath_next, rtl)
                        if head_steps:
                            for _ in range(3):
                                if head_steps:
                                    head_steps.pop(0)()
                    for it in range(NT):
                        nc.tensor.matmul(
                            A_ps[:, it * N:(it + 1) * N],
                            esrow[:, rp, :, it * 128:(it + 1) * 128],
                            eoc[:, :, :],
                            perf_mode=DR,
                            start=(rp == 0), stop=(rp == RT // 2 - 1),
                            skip_group_check=True)
                while head_steps:
                    head_steps.pop(0)()

            def emit_tail_sig(g, hd):
                # sigmoid, mask, row-normalize (scalar/DVE only — emitted right
                # after mid(g) so the chain starts promptly)
                A_ps = hd["A_ps"]
                asig = spool.tile([128, 2 * N], bf16, name="asig", tag="lnx",
                                  bufs=3)
                nc.scalar.activation(asig[:, :], A_ps[:, :], Act.Sigmoid,
                                     scale=1.0 / 32.0)
                A_n = []
                for it in range(NT):
                    am = spool.tile([128, N], bf16, name="am", tag="am")
                    rs = spool.tile([128, 1], f32, name="rs", tag="rs")
                    nc.vector.scalar_tensor_tensor(
                        am[:, :], asig[:, it * N:(it + 1) * N], 1.0,
                        eyemask[it][:, :],
                        op0=Alu.mult, op1=Alu.mult, accum_out=rs[:, :])
                    rr = spool.tile([128, 1], f32, name="rr", tag="rr")
                    nc.vector.reciprocal(rr[:, :], rs[:, :])
                    an = gpool.tile([128, N], bf16, name=f"an{it}", tag=f"an{it}",
                                    bufs=2)
                    nc.vector.tensor_scalar_mul(an[:, :], am[:, :], rr[:, :])
                    A_n.append(an)
                hd["A_n"] = A_n

            def emit_tail_pe(g, hd):
                feat, A_n = hd["feat"], hd["A_n"]
                A_nT = []
                for jt in range(NT):
                    atps = mmps.tile([128, N], bf16, name="atps", tag="mm")
                    for it in range(NT):
                        nc.tensor.transpose(
                            atps[:, it * 128:(it + 1) * 128],
                            A_n[it][:, jt * 128:(jt + 1) * 128], ident_bf16[:, :])
                    anT = gpool.tile([128, N], bf16, name=f"anT{jt}",
                                     tag=f"anT{jt}", bufs=2)
                    nc.scalar.copy(anT[:, :], atps[:, :])
                    A_nT.append(anT)

                # ctxT + h
                ctxT = []
                for half, amat in ((0, A_nT), (1, A_n)):
                    for mt in range(D2 // 128):
                        cps = mmps.tile([128, N], f32, name="cps", tag="mm")
                        for jt in range(NT):
                            nc.tensor.matmul(
                                cps[:, :],
                                feat[jt][:, mt * 128:(mt + 1) * 128], amat[jt][:, :],
                                start=(jt == 0), stop=(jt == NT - 1))
                        ct = gpool.tile([128, N], bf16, name=f"ctxT{half}{mt}",
                                        tag=f"ctxT{half}{mt}", bufs=2)
                        nc.scalar.copy(ct[:, :], cps[:, :])
                        ctxT.append(ct)
                h_pair = tps_pool.tile([128, 2 * DQ], f32, name="h_pair", tag="tps")
                for it in range(NT):
                    for kt in range(DT):
                        nc.tensor.matmul(
                            h_pair[:, it * DQ:(it + 1) * DQ],
                            ctxT[kt][:, it * 128:(it + 1) * 128],
                            tr1_sb[kt][:, :], start=(kt == 0), stop=(kt == DT - 1),
                            skip_group_check=True)
                hd["h_pair"] = h_pair
                if g == GPC - 1:
                    # last graph: preload the sqrt table off the end chain
                    dum = spool.tile([128, 1], f32, name="dsqrt", tag="dsig")
                    nc.scalar.activation(dum[:, :], eps_col[:, :], Act.Sqrt)

            def emit_tail_late(g, hd):
                obj_bf, h_pair = hd["obj_bf"], hd["h_pair"]
                # LayerNorm (ln_g==1, ln_b==0 fast path) via bn_stats + relu
                relu_h = []
                for it in range(NT):
                    h_sl = h_pair[:, it * DQ:(it + 1) * DQ]
                    bns = spool.tile([128, 6], f32, name="bns", tag="bns")
                    nc.vector.bn_stats(bns[:, :], h_sl)
                    mv = spool.tile([128, 2], f32, name="mv", tag="mv")
                    nc.vector.bn_aggr(mv[:, :], bns[:, :])
                    std = spool.tile([128, 1], f32, name="std", tag="std")
                    nc.scalar.activation(std[:, :], mv[:, 1:2], Act.Sqrt,
                                         bias=eps_col[:, :])
                    rstd = spool.tile([128, 1], f32, name="rstd", tag="rstd")
                    nc.vector.reciprocal(rstd[:, :], std[:, :])
                    nmurstd = spool.tile([128, 1], f32, name="nmurstd", tag="nmurstd")
                    nc.vector.scalar_tensor_tensor(
                        nmurstd[:, :], mv[:, 0:1], -1.0, rstd[:, :],
                        op0=Alu.mult, op1=Alu.mult)
                    rh = spool.tile([128, DQ], bf16, name="rh", tag=f"rh{it}", bufs=1)
                    nc.scalar.activation(rh[:, :], h_sl, Act.Relu,
                                         bias=nmurstd[:, :], scale=rstd[:, :])
                    relu_h.append(rh)
                rhT = spool.tile([128, 2, N], bf16, name="rhT", tag="rhT")
                for qt in range(DQ // 128):
                    htps = mmps.tile([128, N], bf16, name="htps", tag="mm")
                    for it in range(NT):
                        nc.tensor.transpose(
                            htps[:, it * 128:(it + 1) * 128],
                            relu_h[it][:, qt * 128:(qt + 1) * 128], ident_bf16[:, :])
                    nc.scalar.copy(rhT[:, qt, :], htps[:, :])

                # nb (bf16) + residual via identity-matmul + relu + store
                for it in range(NT):
                    for fc in range(2):
                        nbh = tps_pool.tile([128, 512], f32, name="nbh", tag="tps")
                        for qt in range(DQ // 128):
                            nc.tensor.matmul(
                                nbh[:, :],
                                rhT[:, qt, it * 128:(it + 1) * 128],
                                tr2_sb[qt][:, fc * 512:(fc + 1) * 512],
                                start=(qt == 0), stop=False)
                        nc.tensor.matmul(
                            nbh[:, :],
                            ident_bf16[:, :],
                            obj_bf[it][:, fc * 512:(fc + 1) * 512],
                            start=False, stop=True)
                        res = spool.tile([128, 512], f32, name="res", tag="res")
                        nc.scalar.activation(res[:, :], nbh[:, :], Act.Relu)
                        nc.sync.dma_start(
                            out_d[g, it * 128:(it + 1) * 128,
                                  fc * 512:(fc + 1) * 512],
                            res[:, :])

            # ================= interleaved emission =================
            # Steady state: ... mid(g,c0..c3), tail_sig(g), head(g+1),
            # mid(g+1,c0), tail_pe(g), mid(g+1,c1), tail_late(g),
            # mid(g+1,c2..c3), tail_sig(g+1), ...  — each tail piece's serial
            # scalar/DVE chain is covered by the next graph's matmul stream.
            hd = emit_head(0, g0)
            prev = None  # graph whose tail_pe/tail_late are pending
            for g in range(GPC):
                emit_gathers(g, hd, 0)
                for rc in range(NCH):
                    emit_qloop(g, hd, rc,
                               gath_next=rc + 1 if rc + 1 < NCH else None)
                    if prev is not None:
                        if rc == 0:
                            emit_tail_pe(prev[0], prev[1])
                        elif rc == 1:
                            emit_tail_late(prev[0], prev[1])
                            prev = None
                emit_tail_sig(g, hd)
                if g + 1 < GPC:
                    nxt = load_proj_inputs(g + 1)
                    nxt = load_mid_inputs(g + 1, nxt)
                    nxt = emit_head(g + 1, nxt)
                else:
                    nxt = None
                prev = (g, hd)
                hd = nxt
            emit_tail_pe(prev[0], prev[1])
            emit_tail_late(prev[0], prev[1])

    nc.compile()
    return nc


_CACHE = {}


def _get_nc():
    if "fast" not in _CACHE:
        _CACHE["fast"] = _build_fast()
    return _CACHE["fast"]


def _reference_numpy(obj_feats, union_feats, ws_w, ws_b, wo_w, wo_b, wu_w, wu_b,
                     w_w, w_b, t3_w, t3_b, tr1_w, tr1_b, ln_g, ln_b, tr2_w, tr2_b,
                     rel_pair_idx):
    """Exact-math fallback for the (unused in practice) nonzero-bias case."""
    outs = []
    n = obj_feats.shape[1]
    eye = 1.0 - np.eye(n, dtype=np.float32)
    sig = lambda x: 1.0 / (1.0 + np.exp(-x))
    for g in range(obj_feats.shape[0]):
        obj, union, pairs = obj_feats[g], union_feats[g], rel_pair_idx[g]
        s = obj[pairs[:, 0]] @ ws_w + ws_b
        o = obj[pairs[:, 1]] @ wo_w + wo_b
        u = union @ wu_w + wu_b
        coeff = ((s * o * u) @ w_w + w_b)[:, 0]
        A = np.zeros((n, n), np.float32)
        np.add.at(A, (pairs[:, 0], pairs[:, 1]), coeff)
        A = sig(A) * eye
        A = A / A.sum(axis=1, keepdims=True)
        feat = np.maximum(obj @ t3_w + t3_b, 0.0)
        ctx = np.concatenate([A @ feat, A.T @ feat], axis=-1)
        h = ctx @ tr1_w + tr1_b
        mu = h.mean(-1, keepdims=True)
        var = ((h - mu) ** 2).mean(-1, keepdims=True)
        h = (h - mu) / np.sqrt(var + LN_EPS) * ln_g + ln_b
        nb = np.maximum(h, 0.0) @ tr2_w + tr2_b
        outs.append(np.maximum(obj + nb, 0.0))
    return np.stack(outs)


def kernel(**inputs) -> np.ndarray:
    obj = np.asarray(inputs["obj_feats"], np.float32)
    union = np.asarray(inputs["union_feats"], np.float32)
    idx = np.asarray(inputs["rel_pair_idx"]).astype(np.int64)
    ws_w = np.asarray(inputs["ws_w"], np.float32)
    ws_b = np.asarray(inputs["ws_b"], np.float32)
    wo_w = np.asarray(inputs["wo_w"], np.float32)
    wo_b = np.asarray(inputs["wo_b"], np.float32)
    wu_w = np.asarray(inputs["wu_w"], np.float32)
    wu_b = np.asarray(inputs["wu_b"], np.float32)
    w_w = np.asarray(inputs["w_w"], np.float32)
    w_b = np.asarray(inputs["w_b"], np.float32)
    t3_w = np.asarray(inputs["t3_w"], np.float32)
    t3_b = np.asarray(inputs["t3_b"], np.float32)
    tr1_w = np.asarray(inputs["tr1_w"], np.float32)
    tr1_b = np.asarray(inputs["tr1_b"], np.float32)
    ln_g = np.asarray(inputs["ln_g"], np.float32)
    ln_b = np.asarray(inputs["ln_b"], np.float32)
    tr2_w = np.asarray(inputs["tr2_w"], np.float32)
    tr2_b = np.asarray(inputs["tr2_b"], np.float32)

    trivial = (not np.any(ws_b) and not np.any(wo_b) and not np.any(wu_b)
               and not np.any(t3_b) and not np.any(tr1_b) and not np.any(tr2_b)
               and not np.any(ln_b) and not np.any(w_b) and np.all(ln_g == 1.0))
    if not trivial:
        return _reference_numpy(obj, union, ws_w, ws_b, wo_w, wo_b, wu_w, wu_b,
                                w_w, w_b, t3_w, t3_b, tr1_w, tr1_b, ln_g, ln_b,
                                tr2_w, tr2_b, idx)

    nc = _get_nc()

    # ---- host-side prep (weight folding, transposes, one-hots) ----
    def pack_dr(w, scale):
        # [D, cols] -> [DT//2, 128, 2, cols] with k = dtp*256 + j*128 + p
        cols = w.shape[1]
        return np.ascontiguousarray(
            (w * scale).reshape(DT // 2, 2, 128, cols)
            .transpose(0, 2, 1, 3).astype(FP8))

    ws8 = pack_dr(ws_w, 64.0)
    wo8 = pack_dr(wo_w, 64.0)
    wu8 = pack_dr((wu_w * w_w[:, 0][None, :]).T, 4096.0)
    t38 = pack_dr(t3_w, 64.0)
    tr1bf = np.ascontiguousarray(tr1_w.astype(BF16))
    tr2bf = np.ascontiguousarray(tr2_w.astype(BF16))
    wb = np.ascontiguousarray(w_b.reshape(1, 1).astype(np.float32))

    objbf = np.ascontiguousarray(obj.astype(BF16))
    # objT8[g, p, dtp, j, n] = obj[g, n, dtp*256 + j*128 + p]
    objT8 = np.ascontiguousarray(
        obj.transpose(0, 2, 1).reshape(B, DT // 2, 2, 128, N)
        .transpose(0, 3, 1, 2, 4).astype(FP8))
    unbf = np.ascontiguousarray(union.astype(BF16))

    # esT8[g, s, p, j, r] = (idx[g, r, s] == j*128 + p)
    tgt = (np.arange(2)[None, :] * 128 + np.arange(128)[:, None])  # [128, 2]
    esT8 = (idx.transpose(0, 2, 1)[:, :, None, None, :]
            == tgt[None, None, :, :, None]).astype(FP8)
    esT8 = np.ascontiguousarray(esT8)
    # esrow[g, p, rp, j, n] = (idx[g, rp*256 + j*128 + p, 0] == n) — DoubleRow
    # pair layout for the A-scatter; ohrow likewise from idx[..., 1]
    ar_n = np.arange(N)
    esrow = (idx[:, :, 0, None] == ar_n).astype(FP8) \
        .reshape(B, RT // 2, 2, 128, N).transpose(0, 3, 1, 2, 4)
    esrow = np.ascontiguousarray(esrow)
    ohrow = (idx[:, :, 1, None] == ar_n).astype(FP8) \
        .reshape(B, RT // 2, 2, 128, N).transpose(0, 3, 1, 2, 4)
    ohrow = np.ascontiguousarray(ohrow)

    in_maps = []
    for c in range(NCORES):
        sl = slice(c * GPC, (c + 1) * GPC)
        in_maps.append({
            "objbf": np.ascontiguousarray(objbf[sl]),
            "objT8": np.ascontiguousarray(objT8[sl]),
            "unbf": np.ascontiguousarray(unbf[sl]),
            "esT8": np.ascontiguousarray(esT8[sl]),
            "esrow": np.ascontiguousarray(esrow[sl]),
            "ohrow": np.ascontiguousarray(ohrow[sl]),
            "ws8": ws8, "wo8": wo8, "wu8": wu8, "t38": t38,
            "tr1bf": tr1bf, "tr2bf": tr2bf, "wb": wb,
        })

    global _last_in_maps
    _last_in_maps = in_maps
    res = bass_utils.run_bass_kernel_spmd(nc, in_maps, core_ids=list(range(NCORES)))
    out = np.concatenate([res.results[c]["out"] for c in range(NCORES)], axis=0)
    return out.astype(np.float32)


_last_in_maps = None


if __name__ == "__main__":
    print("building kernel...")
    _get_nc()
    print("built ok")
